# revision 4
# baseline (speedup 1.0000x reference)
"""Trainium2 Bass kernel for nn_MobileOptimizedSimpleClawMatrix.

Computation (per batch element b):
    vp  = x_v @ Wv.T + bv                     [L, D]
    lp  = x_l @ Wl.T + bl                     [L, D]
    sim = vp @ lp.T                           [L, L]
    attn = softmax(sim, axis=-1)
    av  = attn @ vp                           [L, D]
    al  = attn.T @ lp                         [L, D]
    out = concat([av, al], -1) @ Wo.T + bo    [L, D]

Sharding: batch B=8 across the 8 NeuronCores (data parallel, params
replicated).  Each core runs the full per-batch-element pipeline.

All GEMMs run as fp32r (TF32-class, ~1.6e-4 rounding) at full PE rate.
Softmax runs in fp32 on ACT/DVE.  attn (normalized) and avT are spilled
to DRAM scratch and re-read, which keeps the SBUF working set under the
192KB/partition budget.
"""

import numpy as np

B = 8
L = 2048  # tokens
D = 768  # feature dim
P = 128
NK = D // P  # 6 chunks over feature dim
NT = L // P  # 16 token blocks
NPAIR = NT // 2  # 8 i-block pairs (av accumulates over 256-wide i slices)
NJS = L // 256  # 8 j slices for the al/out phase

_CACHE = {}


def _build_nc():
    from contextlib import ExitStack

    import concourse.bacc as bacc
    import concourse.mybir as mybir
    import concourse.tile as tile
    from concourse.masks import make_identity

    F32 = mybir.dt.float32
    F32R = mybir.dt.float32r
    Exp = mybir.ActivationFunctionType.Exp
    Identity = mybir.ActivationFunctionType.Identity
    X = mybir.AxisListType.X

    nc = bacc.Bacc("TRN2", target_bir_lowering=False, debug=False, num_devices=B)

    # ---- DRAM I/O (per core; host pre-transposes x and W) ----
    xvT = nc.dram_tensor("xvT", [D, L], F32, kind="ExternalInput")
    xlT = nc.dram_tensor("xlT", [D, L], F32, kind="ExternalInput")
    wvT = nc.dram_tensor("wvT", [D, D], F32, kind="ExternalInput")  # Wv.T [d, e]
    wlT = nc.dram_tensor("wlT", [D, D], F32, kind="ExternalInput")  # Wl.T [d, e]
    woT = nc.dram_tensor("woT", [2 * D, D], F32, kind="ExternalInput")  # Wo.T [c, e]
    bv = nc.dram_tensor("bv", [D], F32, kind="ExternalInput")
    bl = nc.dram_tensor("bl", [D], F32, kind="ExternalInput")
    bo = nc.dram_tensor("bo", [D], F32, kind="ExternalInput")
    out = nc.dram_tensor("out", [L, D], F32, kind="ExternalOutput")

    # d-major views of DRAM scratch for chunked access
    xvT_v = xvT[:].rearrange("(k p) t -> p k t", p=P)
    xlT_v = xlT[:].rearrange("(k p) t -> p k t", p=P)
    wvT_v = wvT[:].rearrange("(k p) e -> p k e", p=P)
    wlT_v = wlT[:].rearrange("(k p) e -> p k e", p=P)
    woT_v = woT[:].rearrange("(k p) e -> p k e", p=P)

    with ExitStack() as ctx:
        tc = ctx.enter_context(tile.TileContext(nc))

        dram = ctx.enter_context(tc.tile_pool(name="dram", bufs=1, space="DRAM"))
        escratch = dram.tile([L, L], F32R)  # normalized attn
        avT_dram = dram.tile([D, L], F32R)  # aligned_vision^T
        escr_v = escratch[:].rearrange("(c p) j -> p c j", p=P)
        avT_v = avT_dram[:].rearrange("(m p) t -> p m t", p=P)

        # ---- persistent pools ----
        const = ctx.enter_context(tc.tile_pool(name="const", bufs=1))
        vp_pool = ctx.enter_context(tc.tile_pool(name="vp", bufs=1))
        lpT_pool = ctx.enter_context(tc.tile_pool(name="lpT", bufs=1))
        wpool = ctx.enter_context(tc.tile_pool(name="wts", bufs=1))

        ident_f = const.tile([P, P], F32)
        make_identity(nc, ident_f[:])
        ident = const.tile([P, P], F32R)
        nc.vector.tensor_copy(ident[:], ident_f[:])
        ones_f = const.tile([1, P], F32)
        nc.gpsimd.memset(ones_f[:], 1.0)
        ones1 = const.tile([1, P], F32R)
        nc.vector.tensor_copy(ones1[:], ones_f[:])
        bvr = const.tile([1, D], F32R)
        nc.gpsimd.dma_start(bvr[:], bv[:].unsqueeze(0))
        bor = const.tile([1, D], F32R)
        nc.gpsimd.dma_start(bor[:], bo[:].unsqueeze(0))
        bl_col = const.tile([P, NK], F32)
        nc.sync.dma_start(bl_col[:], bl[:].rearrange("(o p) -> p o", p=P))

        # persistent arrays
        vp_t = vp_pool.tile([P, NT, D], F32R, tag="vpslot")  # vp[i, d] token-major
        lpT_t = lpT_pool.tile([P, NK, L], F32R)  # lp^T [d, t]

        # weights: wv+wl share one 12-chunk slot, recycled for woT later
        w12 = wpool.tile([P, 2 * NK, D], F32R, tag="w12")  # [:, 0:6]=WvT, [:, 6:12]=WlT
        nc.gpsimd.dma_start(w12[:, 0:NK, :], wvT_v)
        nc.gpsimd.dma_start(w12[:, NK : 2 * NK, :], wlT_v)

        # ================= Phase A: projections =================
        with ExitStack() as actx:
            xv_pool = actx.enter_context(tc.tile_pool(name="xv", bufs=3))
            xl_pool = actx.enter_context(tc.tile_pool(name="xl", bufs=2))
            pa_vp = actx.enter_context(tc.tile_pool(name="pa_vp", bufs=2, space="PSUM"))
            pa_lp = actx.enter_context(tc.tile_pool(name="pa_lp", bufs=2, space="PSUM"))

            for g in range(4):  # groups of 4 token blocks
                for tb in range(4 * g, 4 * g + 4):
                    xvb = xv_pool.tile([P, NK, P], F32R, tag="xvb")
                    nc.gpsimd.dma_start(xvb[:], xvT_v[:, :, tb * P : (tb + 1) * P])
                    vps = pa_vp.tile([P, D], F32, tag="vps")
                    for k in range(NK):
                        nc.tensor.matmul(
                            vps[:, 0:512], xvb[:, k, :], w12[:, k, 0:512],
                            start=(k == 0), stop=False,
                        )
                        nc.tensor.matmul(
                            vps[:, 512:768], xvb[:, k, :], w12[:, k, 512:768],
                            start=(k == 0), stop=False,
                        )
                    nc.tensor.matmul(
                        vps[:, 0:512], ones1[:], bvr[:, 0:512], start=False, stop=True
                    )
                    nc.tensor.matmul(
                        vps[:, 512:768], ones1[:], bvr[:, 512:768],
                        start=False, stop=True,
                    )
                    nc.vector.tensor_copy(vp_t[:, tb, :], vps[:])

                # lpT for this 512-token slice
                xlg = xl_pool.tile([P, NK, 512], F32R, tag="xlg")
                nc.gpsimd.dma_start(xlg[:], xlT_v[:, :, g * 512 : (g + 1) * 512])
                for me in range(NK):
                    lps = pa_lp.tile([P, 512], F32, tag="lps")
                    for k in range(NK):
                        nc.tensor.matmul(
                            lps[:], w12[:, NK + k, me * P : (me + 1) * P],
                            xlg[:, k, :],
                            start=(k == 0), stop=(k == NK - 1),
                        )
                    nc.scalar.activation(
                        lpT_t[:, me, g * 512 : (g + 1) * 512], lps[:],
                        Identity, bias=bl_col[:, me : me + 1], scale=1.0,
                    )

        # ================= Phase B: attention + av =================
        with ExitStack() as bctx:
            vpT_pool = bctx.enter_context(tc.tile_pool(name="vpT", bufs=2))
            attn_pool = bctx.enter_context(tc.tile_pool(name="attn", bufs=2))
            attnT_pool = bctx.enter_context(tc.tile_pool(name="attnT", bufs=2))
            avev_pool = bctx.enter_context(tc.tile_pool(name="avev", bufs=2))
            stat_pool = bctx.enter_context(tc.tile_pool(name="stat", bufs=4))
            pb_sim = bctx.enter_context(
                tc.tile_pool(name="pb_sim", bufs=1, space="PSUM")
            )
            pb_tr = bctx.enter_context(tc.tile_pool(name="pb_tr", bufs=2, space="PSUM"))
            pb_av = bctx.enter_context(tc.tile_pool(name="pb_av", bufs=2, space="PSUM"))

            for pair in range(NPAIR):
                attnT = attnT_pool.tile([P, NT, 256], F32R, tag="attnT")
                for half in range(2):
                    iblk = 2 * pair + half
                    # vpT slices for this block
                    vpTb = vpT_pool.tile([P, NK, P], F32R, tag="vpTb")
                    for k in range(NK):
                        ptr = pb_tr.tile([P, P], F32R, tag="ptr")
                        nc.tensor.transpose(
                            ptr[:], vp_t[:, iblk, k * P : (k + 1) * P], ident[:]
                        )
                        nc.vector.tensor_copy(vpTb[:, k, :], ptr[:])
                    # sim row block [128, 2048]
                    sim = pb_sim.tile([P, L], F32, tag="sim")
                    for ns in range(4):
                        for k in range(NK):
                            nc.tensor.matmul(
                                sim[:, ns * 512 : (ns + 1) * 512],
                                vpTb[:, k, :],
                                lpT_t[:, k, ns * 512 : (ns + 1) * 512],
                                start=(k == 0), stop=(k == NK - 1),
                            )
                    # softmax (rows)
                    negm = stat_pool.tile([P, 1], F32, tag="negm")
                    nc.vector.reduce_max(negm[:], sim[:], axis=X, negate=True)
                    attn = attn_pool.tile([P, L], F32R, tag="attn")
                    z = stat_pool.tile([P, 1], F32, tag="z")
                    nc.scalar.activation(
                        attn[:], sim[:], Exp, bias=negm[:], scale=1.0, accum_out=z[:]
                    )
                    rz = stat_pool.tile([P, 1], F32, tag="rz")
                    nc.vector.reciprocal(rz[:], z[:])
                    nc.vector.tensor_scalar_mul(attn[:], attn[:], rz[:])
                    # spill normalized attn row block
                    nc.sync.dma_start(
                        escratch[iblk * P : (iblk + 1) * P, :], attn[:]
                    )
                    # transpose into attnT[:, jc, half*128:...]
                    for jc in range(NT):
                        ptr = pb_tr.tile([P, P], F32R, tag="ptr")
                        nc.tensor.transpose(
                            ptr[:], attn[:, jc * P : (jc + 1) * P], ident[:]
                        )
                        nc.scalar.copy(
                            attnT[:, jc, half * P : (half + 1) * P], ptr[:]
                        )
                # avT[:, pair slice] = sum_j vp[j,:]^T x attnT
                avev = avev_pool.tile([P, NK, 256], F32R, tag="avev")
                for md in range(NK):
                    avp = pb_av.tile([P, 256], F32, tag="avp")
                    for jc in range(NT):
                        nc.tensor.matmul(
                            avp[:],
                            vp_t[:, jc, md * P : (md + 1) * P],
                            attnT[:, jc, :],
                            start=(jc == 0), stop=(jc == NT - 1),
                        )
                    nc.vector.tensor_copy(avev[:, md, :], avp[:])
                nc.sync.dma_start(
                    avT_v[:, :, pair * 256 : (pair + 1) * 256], avev[:]
                )

        # ================= Phase C/D: al + output projection =================
        with ExitStack() as cctx:
            lp_t = vp_pool.tile([P, NT, D], F32R, tag="vpslot")  # reuse vp slot
            col_pool = cctx.enter_context(tc.tile_pool(name="col", bufs=2))
            alT_pool = cctx.enter_context(tc.tile_pool(name="alT", bufs=2))
            avtt_pool = cctx.enter_context(tc.tile_pool(name="avtt", bufs=2))
            outsb_pool = cctx.enter_context(tc.tile_pool(name="outsb", bufs=2))
            pc_tr = cctx.enter_context(tc.tile_pool(name="pc_tr", bufs=2, space="PSUM"))
            pc_al = cctx.enter_context(tc.tile_pool(name="pc_al", bufs=2, space="PSUM"))
            pd_out = cctx.enter_context(
                tc.tile_pool(name="pd_out", bufs=2, space="PSUM")
            )

            # derive lp token-major from lpT (96 transposes)
            for ic in range(NT):
                for k in range(NK):
                    ptr = pc_tr.tile([P, P], F32R, tag="ptr")
                    nc.tensor.transpose(
                        ptr[:], lpT_t[:, k, ic * P : (ic + 1) * P], ident[:]
                    )
                    nc.scalar.copy(lp_t[:, ic, k * P : (k + 1) * P], ptr[:])

            # woT into the recycled weight slot
            woT_t = wpool.tile([P, 2 * NK, D], F32R, tag="w12")
            nc.gpsimd.dma_start(woT_t[:], woT_v)

            for js in range(NJS):
                colt = col_pool.tile([P, NT, 256], F32R, tag="colt")
                nc.sync.dma_start(colt[:], escr_v[:, :, js * 256 : (js + 1) * 256])
                alT = alT_pool.tile([P, NK, 256], F32R, tag="alT")
                for md in range(NK):
                    alp = pc_al.tile([P, 256], F32, tag="alp")
                    for ic in range(NT):
                        nc.tensor.matmul(
                            alp[:],
                            lp_t[:, ic, md * P : (md + 1) * P],
                            colt[:, ic, :],
                            start=(ic == 0), stop=(ic == NT - 1),
                        )
                    nc.vector.tensor_copy(alT[:, md, :], alp[:])
                # output projection for the 2 token tiles in this slice
                for half in range(2):
                    tt = 2 * js + half
                    avtt = avtt_pool.tile([P, NK, P], F32R, tag="avtt")
                    nc.sync.dma_start(avtt[:], avT_v[:, :, tt * P : (tt + 1) * P])
                    ops = pd_out.tile([P, D], F32, tag="ops")
                    for kc in range(2 * NK):
                        lhsT = (
                            avtt[:, kc, :]
                            if kc < NK
                            else alT[:, kc - NK, half * P : (half + 1) * P]
                        )
                        nc.tensor.matmul(
                            ops[:, 0:512], lhsT, woT_t[:, kc, 0:512],
                            start=(kc == 0), stop=False,
                        )
                        nc.tensor.matmul(
                            ops[:, 512:768], lhsT, woT_t[:, kc, 512:768],
                            start=(kc == 0), stop=False,
                        )
                    nc.tensor.matmul(
                        ops[:, 0:512], ones1[:], bor[:, 0:512], start=False, stop=True
                    )
                    nc.tensor.matmul(
                        ops[:, 512:768], ones1[:], bor[:, 512:768],
                        start=False, stop=True,
                    )
                    outsb = outsb_pool.tile([P, D], F32, tag="outsb")
                    nc.vector.tensor_copy(outsb[:], ops[:])
                    nc.sync.dma_start(out[tt * P : (tt + 1) * P, :], outsb[:])

    nc.compile()
    return nc


def kernel(
    vision_features, language_features, Wv, bv, Wl, bl, Wo, bo
) -> np.ndarray:
    from concourse.bass_utils import run_bass_kernel_spmd

    nc = _CACHE.get("nc")
    if nc is None:
        nc = _build_nc()
        _CACHE["nc"] = nc

    wvT = np.ascontiguousarray(np.asarray(Wv, dtype=np.float32).T)
    wlT = np.ascontiguousarray(np.asarray(Wl, dtype=np.float32).T)
    woT = np.ascontiguousarray(np.asarray(Wo, dtype=np.float32).T)
    bv = np.asarray(bv, dtype=np.float32)
    bl = np.asarray(bl, dtype=np.float32)
    bo = np.asarray(bo, dtype=np.float32)
    vision_features = np.asarray(vision_features, dtype=np.float32)
    language_features = np.asarray(language_features, dtype=np.float32)

    in_maps = []
    for b in range(B):
        in_maps.append(
            {
                "xvT": np.ascontiguousarray(vision_features[b].T),
                "xlT": np.ascontiguousarray(language_features[b].T),
                "wvT": wvT,
                "wlT": wlT,
                "woT": woT,
                "bv": bv,
                "bl": bl,
                "bo": bo,
            }
        )

    res = run_bass_kernel_spmd(nc, in_maps, list(range(B)))
    return np.stack([res.results[b]["out"] for b in range(B)]).astype(np.float32)
